# revision 8
# baseline (speedup 1.0000x reference)
"""Cross-attention kernel for 8 Trainium2 NeuronCores.

Problem (hardcoded): x [4,4096,512], context [4,1024,768], 8 heads x 64,
inner 512. out = softmax((x@Wq)(ctx@Wk)^T / 8) @ (ctx@Wv) @ Wo + bo.

Sharding: 8 cores = 4 batches x 2 head-groups (4 heads each).
Core c handles batch b=c//2, heads [4g, 4g+4) with g=c%2:
  - Wq/Wk/Wv column-sliced, Wo row-sliced (tensor parallel over heads)
  - each core emits a partial [4096, 512]; host sums the two head-group
    partials per batch and adds bo.

Device-side layout:
  - host pre-transposes x/context so all projections contract naturally
    (feature dim on partitions); no on-device transposes.
  - qT/kT hold head pairs stacked on partitions (rows 0-63 = even head
    of the pair, 64-127 = odd head) so the K=64 score matmuls occupy
    disjoint PE row-groups (LDWEIGHTS overlaps in-flight MATMULs).
  - scores are built transposed [keys, q]; exp (fp16) feeds the AV
    matmul directly as the moving operand.
  - V stationaries are zero-padded to the full 128 columns per head:
    head j=0 occupies cols 0-63 with a ones column at col 64 (softmax
    denominator rides the AV matmul); head j=1 occupies cols 64-127
    with ones at col 63. This lands the odd head's attn rows directly
    on partitions 64-127 (no partition-shift DMA) and enables FWL.
  - softmax normalize is DMA-free: both denominators land on adjacent
    partitions (63/64), reciprocal on DVE, then a K=2 matmul against a
    0/1 selector broadcasts 1/denom across partitions; gpsimd applies
    the scale into fp16 attnT tiles.
  - PE stream is software-pipelined: scores(kc+2) are emitted before
    AV(kc) so the PE never stalls on the exp of the current tile (and
    the HAM clock-gate stays warm at 2.4 GHz).
"""

import os
import sys

for _p in ("/opt/trn_rl_repo", "/root/.axon_site/_ro/trn_rl_repo"):
    if os.path.isdir(_p) and _p not in sys.path:
        sys.path.append(_p)

import numpy as np

F16_NP = np.float16

import concourse.bass as bass  # noqa: E402
import concourse.mybir as mybir  # noqa: E402
import concourse.tile as tile  # noqa: E402
from concourse import bacc  # noqa: E402
from concourse import bass_utils  # noqa: E402

P = 128
B = 4
NQ = 4096  # queries per batch
DX = 512  # x feature dim (4 chunks of 128)
NC = 1024  # context length (8 key chunks of 128)
DC = 768  # context feature dim (6 chunks of 128)
DH = 64  # head dim
HPC = 4  # heads per core
COLS = HPC * DH  # 256 = per-core slice of the inner dim
DOUT = 512  # output dim

DXC = DX // P  # 4
DCC = DC // P  # 6
KC = NC // P  # 8 key chunks
NQT = NQ // 512  # 8 query tiles of 512
NQB = NQ // 1024  # 4 query blocks of 1024

F32 = mybir.dt.float32
F16 = mybir.dt.float16
EXP = mybir.ActivationFunctionType.Exp
SCALE = DH**-0.5  # 0.125, folded into the exp activation's scale


def _emit(tc, nc, xT, ctxT, wq, wk, wv, wo, out):
    with (
        tc.tile_pool(name="consts", bufs=1) as consts,
        tc.tile_pool(name="xstream", bufs=2) as xstream,
        tc.tile_pool(name="qtpool", bufs=4) as qtpool,
        tc.tile_pool(name="etile", bufs=4) as etile,
        tc.tile_pool(name="atpool", bufs=4) as atpool,
        tc.tile_pool(name="norm", bufs=2) as norm,
        tc.tile_pool(name="ost", bufs=2) as ost,
    ):
        # ---- weights + context into SBUF (feature dim on partitions) ----
        wq_sb = consts.tile([P, DXC, COLS], F16, tag="wq", name="wq_sb")
        wk_sb = consts.tile([P, DCC, COLS], F16, tag="wk", name="wk_sb")
        wv_sb = consts.tile([P, DCC, COLS], F16, tag="wv", name="wv_sb")
        wo_sb = consts.tile([P, 2, DOUT], F16, tag="wo", name="wo_sb")
        ctx_pool_cm = tc.tile_pool(name="ctxpool", bufs=1)
        ctx_pool = ctx_pool_cm.__enter__()
        ctxT_sb = ctx_pool.tile([P, DCC, NC], F16, tag="ctxT", name="ctxT_sb")
        nc.sync.dma_start(wk_sb[:], wk.rearrange("(c p) n -> p c n", p=P))
        nc.sync.dma_start(ctxT_sb[:], ctxT.rearrange("(c p) n -> p c n", p=P))
        nc.sync.dma_start(wv_sb[:], wv.rearrange("(c p) n -> p c n", p=P))
        nc.sync.dma_start(wq_sb[:], wq.rearrange("(c p) n -> p c n", p=P))
        nc.sync.dma_start(wo_sb[:], wo.rearrange("(c p) n -> p c n", p=P))

        # all-ones rows at partitions 0 and 64 for the K=1 reciprocal
        # broadcast matmuls (matmul operands must base at 0/32/64)
        sel = consts.tile([P, P], F16, tag="sel", name="sel")
        nc.gpsimd.memset(sel[0:1, :], 1.0)
        nc.gpsimd.memset(sel[64:65, :], 1.0)

        # V stationaries, zero-padded to 128 cols, ones columns for denoms:
        # j=0 ones at col 64 (denom -> acc partition 64), j=1 ones at col 0
        # (denom -> acc partition 0)
        vst = consts.tile([P, KC, 2, 2, P], F16, tag="vst", name="vst")
        nc.vector.memset(vst[:], 0.0)
        nc.gpsimd.memset(vst[:, :, :, 0, DH : DH + 1], 1.0)
        nc.gpsimd.memset(vst[:, :, :, 1, 0:1], 1.0)

        # PSUM pools: 3x2 banks work ring + 2x1 bank attn accumulators
        work_cm = tc.tile_pool(name="work", bufs=3, space="PSUM")
        work = work_cm.__enter__()
        attnps_cm = tc.tile_pool(name="attnps", bufs=2, space="PSUM")
        attnps = attnps_cm.__enter__()

        def work_tile(name):
            return work.tile([P, 2, 512], F32, tag="work", name=name)

        # ---- K^T projection: kT[pair][2*64 head dims, 1024 keys] ----
        kT_sb = [
            consts.tile([P, NC], F16, tag=f"kT{p}", name=f"kT{p}") for p in range(2)
        ]
        for p in range(2):
            acc = work_tile("kproj_acc")
            for h in range(2):
                for ch in range(DCC):
                    nc.tensor.matmul(
                        acc[:, h, :],
                        wk_sb[:, ch, p * P : (p + 1) * P],
                        ctxT_sb[:, ch, h * 512 : (h + 1) * 512],
                        start=(ch == 0),
                        stop=(ch == DCC - 1),
                    )
            nc.vector.tensor_copy(kT_sb[p][:], acc.rearrange("p a b -> p (a b)"))

        # ---- Q^T projection per 1024-q block (emitted 1 block ahead) ----
        qT_sb = {}

        def emit_qproj(qb):
            xt = xstream.tile([P, DXC, 1024], F16, tag="xt", name="xt")
            nc.sync.dma_start(
                xt[:],
                xT.rearrange("(c p) q -> p c q", p=P)[
                    :, :, qb * 1024 : (qb + 1) * 1024
                ],
            )
            for p in range(2):
                acc = work_tile("qproj_acc")
                for h in range(2):
                    for ch in range(DXC):
                        nc.tensor.matmul(
                            acc[:, h, :],
                            wq_sb[:, ch, p * P : (p + 1) * P],
                            xt[:, ch, h * 512 : (h + 1) * 512],
                            start=(ch == 0),
                            stop=(ch == DXC - 1),
                        )
                qt_t = qtpool.tile([P, 1024], F16, tag="qT", name=f"qT{p}_{qb}")
                qT_sb[(p, qb)] = qt_t
                nc.vector.tensor_copy(qt_t[:], acc.rearrange("p a b -> p (a b)"))

        emit_qproj(0)

        # ---- V projection: keys on partitions, scattered into vst ----
        for kc in range(KC):
            acc = work_tile("vproj_acc")
            av = acc[:, 0, 0:COLS].rearrange("p (a b c) -> p a b c", a=2, b=2)
            for ch in range(DCC):
                nc.tensor.matmul(
                    acc[:, 0, 0:COLS],
                    ctxT_sb[:, ch, kc * P : (kc + 1) * P],
                    wv_sb[:, ch, :],
                    start=(ch == 0),
                    stop=(ch == DCC - 1),
                )
            nc.vector.tensor_copy(vst[:, kc, :, 0, 0:DH], av[:, :, 0, :])
            nc.vector.tensor_copy(vst[:, kc, :, 1, DH:P], av[:, :, 1, :])

        emit_qproj(1)

        ctx_pool_cm.__exit__(None, None, None)

        # ---- attention + output projection, per 512-query tile ----
        attnT_all = {}

        def outproj(qt):
            for sub in range(4):
                o = work_tile("oproj_acc")[:, 0, :]
                for p in range(2):
                    nc.tensor.matmul(
                        o[:],
                        attnT_all[(p, qt)][:, sub * P : (sub + 1) * P],
                        wo_sb[:, p, :],
                        start=(p == 0),
                        stop=(p == 1),
                    )
                ostage = ost.tile([P, DOUT], F32, tag="ostage", name="ostage_t")
                nc.vector.tensor_copy(ostage[:], o[:])
                row = qt * 512 + sub * P
                nc.gpsimd.dma_start(out[row : row + P, :], ostage[:])

        def attn_pair(qt, p):
            qb, qh = qt // 2, qt % 2
            qmv = qT_sb[(p, qb)]
            accA = attnps.tile([P, 512], F32, tag="attnacc", name="accA")
            accB = attnps.tile([P, 512], F32, tag="attnacc", name="accB")
            ex_t = {}

            def emit_sc(kc):
                sc = work_tile("scores_ps")
                for j in range(2):
                    nc.tensor.matmul(
                        sc[:, j, :],
                        kT_sb[p][j * DH : (j + 1) * DH, kc * P : (kc + 1) * P],
                        qmv[j * DH : (j + 1) * DH, qh * 512 : (qh + 1) * 512],
                        start=True,
                        stop=True,
                    )
                ex = etile.tile([P, 2, 512], F16, tag="exp", name="exp_sb")
                nc.scalar.activation(ex[:], sc[:], EXP, scale=SCALE)
                ex_t[kc] = ex

            def emit_av(kc):
                for j in range(2):
                    nc.tensor.matmul(
                        accA[:] if j == 0 else accB[:],
                        vst[:, kc, p, j, :],
                        ex_t[kc][:, j, :],
                        start=(kc == 0),
                        stop=(kc == KC - 1),
                    )

            emit_sc(0)
            emit_sc(1)
            for kc in range(KC):
                if kc + 2 < KC:
                    emit_sc(kc + 2)
                emit_av(kc)

            # ---- DMA-free softmax normalize ----
            dt = norm.tile([P, 512], F32, tag="den", name="den_t")
            nc.vector.tensor_copy(dt[0:1, :], accB[0:1, :])
            nc.vector.tensor_copy(dt[DH : DH + 1, :], accA[DH : DH + 1, :])
            nc.vector.reciprocal(dt[0:1, :], dt[0:1, :])
            nc.vector.reciprocal(dt[DH : DH + 1, :], dt[DH : DH + 1, :])
            d16 = norm.tile([P, 512], F16, tag="den16", name="den16_t")
            nc.vector.tensor_copy(d16[0:1, :], dt[0:1, :])
            nc.vector.tensor_copy(d16[DH : DH + 1, :], dt[DH : DH + 1, :])
            rb = work_tile("rbcast")[:, 0, :]
            nc.tensor.matmul(
                rb[0:DH, :],
                sel[DH : DH + 1, 0:DH],
                d16[DH : DH + 1, :],
                start=True,
                stop=True,
            )
            nc.tensor.matmul(
                rb[DH:P, :],
                sel[0:1, 0:DH],
                d16[0:1, :],
                start=True,
                stop=True,
            )
            rb_sb = norm.tile([P, 512], F16, tag="rbsb", name="rb_sb")
            nc.vector.tensor_copy(rb_sb[:], rb[:])
            at_t = atpool.tile([P, 512], F16, tag="attnT", name=f"attnT{p}_{qt}")
            attnT_all[(p, qt)] = at_t
            nc.vector.tensor_mul(at_t[0:DH, :], accA[0:DH, :], rb_sb[0:DH, :])
            nc.vector.tensor_mul(at_t[DH:P, :], accB[DH:P, :], rb_sb[DH:P, :])

        for qt in range(NQT):
            attn_pair(qt, 0)
            attn_pair(qt, 1)
            if qt % 2 == 1 and qt // 2 + 2 < NQB:
                emit_qproj(qt // 2 + 2)
            if qt >= 1:
                outproj(qt - 1)
        outproj(NQT - 1)
        attnps_cm.__exit__(None, None, None)
        work_cm.__exit__(None, None, None)


def _build():
    nc = bacc.Bacc(
        "TRN2", target_bir_lowering=False, debug=False, enable_asserts=False
    )
    xT = nc.dram_tensor("xT", [DX, NQ], F16, kind="ExternalInput").ap()
    ctxT = nc.dram_tensor("ctxT", [DC, NC], F16, kind="ExternalInput").ap()
    wq = nc.dram_tensor("wq", [DX, COLS], F16, kind="ExternalInput").ap()
    wk = nc.dram_tensor("wk", [DC, COLS], F16, kind="ExternalInput").ap()
    wv = nc.dram_tensor("wv", [DC, COLS], F16, kind="ExternalInput").ap()
    wo = nc.dram_tensor("wo", [COLS, DOUT], F16, kind="ExternalInput").ap()
    out = nc.dram_tensor("out", [NQ, DOUT], F32, kind="ExternalOutput").ap()
    with tile.TileContext(nc) as tc:
        _emit(tc, nc, xT, ctxT, wq, wk, wv, wo, out)
    nc.compile()
    return nc


_NC = None


def _get_nc():
    global _NC
    if _NC is None:
        _NC = _build()
    return _NC


def _in_maps(x, context, Wq, Wk, Wv, Wo):
    maps = []
    for c in range(8):
        b, g = c // 2, c % 2
        cs = slice(g * COLS, (g + 1) * COLS)
        maps.append(
            {
                "xT": np.ascontiguousarray(x[b].T.astype(F16_NP)),
                "ctxT": np.ascontiguousarray(context[b].T.astype(F16_NP)),
                "wq": np.ascontiguousarray(Wq[:, cs].astype(F16_NP)),
                "wk": np.ascontiguousarray(Wk[:, cs].astype(F16_NP)),
                "wv": np.ascontiguousarray(Wv[:, cs].astype(F16_NP)),
                "wo": np.ascontiguousarray(Wo[cs, :].astype(F16_NP)),
            }
        )
    return maps


def _execute(in_maps, **kw):
    return bass_utils.run_bass_kernel_spmd(
        _get_nc(), in_maps, core_ids=list(range(8)), **kw
    )


def kernel(x, context, Wq, Wk, Wv, Wo, bo):
    x = np.asarray(x, np.float32)
    context = np.asarray(context, np.float32)
    Wq = np.asarray(Wq, np.float32)
    Wk = np.asarray(Wk, np.float32)
    Wv = np.asarray(Wv, np.float32)
    Wo = np.asarray(Wo, np.float32)
    bo = np.asarray(bo, np.float32)
    res = _execute(_in_maps(x, context, Wq, Wk, Wv, Wo))
    parts = [r["out"] for r in res.results]
    out = np.empty((B, NQ, DOUT), np.float32)
    for b in range(B):
        out[b] = parts[2 * b] + parts[2 * b + 1] + bo[None, :]
    return out


# revision 11
# speedup vs baseline: 1.3574x; 1.3574x over previous
"""Cross-attention kernel for 8 Trainium2 NeuronCores.

Problem (hardcoded): x [4,4096,512], context [4,1024,768], 8 heads x 64,
inner 512. out = softmax((x@Wq)(ctx@Wk)^T / 8) @ (ctx@Wv) @ Wo + bo.

Sharding: 8 cores = 4 batches x 2 head-groups (4 heads each).
Core c handles batch b=c//2, heads [4g, 4g+4) with g=c%2:
  - Wq/Wk/Wv column-sliced, Wo row-sliced (tensor parallel over heads)
  - each core emits a partial [4096, 512]; host sums the two head-group
    partials per batch and adds bo.

Device-side layout:
  - host pre-transposes x/context so all projections contract naturally
    (feature dim on partitions); no on-device transposes.
  - qT/kT hold head pairs stacked on partitions (rows 0-63 = even head
    of the pair, 64-127 = odd head) so the K=64 score matmuls occupy
    disjoint PE row-groups (LDWEIGHTS overlaps in-flight MATMULs).
  - scores are built transposed [keys, q]; exp (fp16) feeds the AV
    matmul directly as the moving operand.
  - V stationaries are zero-padded to the full 128 columns per head:
    head j=0 occupies cols 0-63 with a ones column at col 64 (softmax
    denominator rides the AV matmul); head j=1 occupies cols 64-127
    with ones at col 63. This lands the odd head's attn rows directly
    on partitions 64-127 (no partition-shift DMA) and enables FWL.
  - softmax normalize is DMA-free: both denominators land on adjacent
    partitions (63/64), reciprocal on DVE, then a K=2 matmul against a
    0/1 selector broadcasts 1/denom across partitions; gpsimd applies
    the scale into fp16 attnT tiles.
  - PE stream is software-pipelined: scores(kc+2) are emitted before
    AV(kc) so the PE never stalls on the exp of the current tile (and
    the HAM clock-gate stays warm at 2.4 GHz).
"""

import os
import sys

for _p in ("/opt/trn_rl_repo", "/root/.axon_site/_ro/trn_rl_repo"):
    if os.path.isdir(_p) and _p not in sys.path:
        sys.path.append(_p)

import numpy as np

F16_NP = np.float16

import concourse.bass as bass  # noqa: E402
import concourse.mybir as mybir  # noqa: E402
import concourse.tile as tile  # noqa: E402
from concourse import bacc  # noqa: E402
from concourse import bass_utils  # noqa: E402

P = 128
B = 4
NQ = 4096  # queries per batch
DX = 512  # x feature dim (4 chunks of 128)
NC = 1024  # context length (8 key chunks of 128)
DC = 768  # context feature dim (6 chunks of 128)
DH = 64  # head dim
HPC = 4  # heads per core
COLS = HPC * DH  # 256 = per-core slice of the inner dim
DOUT = 512  # output dim

DXC = DX // P  # 4
DCC = DC // P  # 6
KC = NC // P  # 8 key chunks
NQT = NQ // 512  # 8 query tiles of 512
NQB = NQ // 1024  # 4 query blocks of 1024

F32 = mybir.dt.float32
F16 = mybir.dt.float16
EXP = mybir.ActivationFunctionType.Exp
SCALE = DH**-0.5  # 0.125, folded into the exp activation's scale


def _emit(tc, nc, xT, ctxT, wq, wk, wv, wo, out):
    with (
        tc.tile_pool(name="consts", bufs=1) as consts,
        tc.tile_pool(name="xstream", bufs=2) as xstream,
        tc.tile_pool(name="qtpool", bufs=4) as qtpool,
        tc.tile_pool(name="etile", bufs=4) as etile,
        tc.tile_pool(name="atpool", bufs=4) as atpool,
        tc.tile_pool(name="norm", bufs=2) as norm,
        tc.tile_pool(name="ost", bufs=2) as ost,
    ):
        # ---- weights + context into SBUF (feature dim on partitions) ----
        wq_sb = consts.tile([P, DXC, COLS], F16, tag="wq", name="wq_sb")
        wk_sb = consts.tile([P, DCC, COLS], F16, tag="wk", name="wk_sb")
        wv_sb = consts.tile([P, DCC, COLS], F16, tag="wv", name="wv_sb")
        wo_sb = consts.tile([P, 2, DOUT], F16, tag="wo", name="wo_sb")
        ctx_pool_cm = tc.tile_pool(name="ctxpool", bufs=1)
        ctx_pool = ctx_pool_cm.__enter__()
        ctxT_sb = ctx_pool.tile([P, DCC, NC], F16, tag="ctxT", name="ctxT_sb")
        nc.sync.dma_start(wk_sb[:], wk.rearrange("(c p) n -> p c n", p=P))
        nc.sync.dma_start(ctxT_sb[:], ctxT.rearrange("(c p) n -> p c n", p=P))
        nc.sync.dma_start(wv_sb[:], wv.rearrange("(c p) n -> p c n", p=P))
        nc.sync.dma_start(wq_sb[:], wq.rearrange("(c p) n -> p c n", p=P))
        nc.sync.dma_start(wo_sb[:], wo.rearrange("(c p) n -> p c n", p=P))

        # all-ones rows at partitions 0 and 64 for the K=1 reciprocal
        # broadcast matmuls (matmul operands must base at 0/32/64)
        sel = consts.tile([P, P], F16, tag="sel", name="sel")
        nc.gpsimd.memset(sel[0:1, :], 1.0)
        nc.gpsimd.memset(sel[64:65, :], 1.0)

        # V stationaries, zero-padded to 128 cols, ones columns for denoms:
        # j=0 ones at col 64 (denom -> acc partition 64), j=1 ones at col 0
        # (denom -> acc partition 0)
        vst = consts.tile([P, KC, 2, 2, P], F16, tag="vst", name="vst")
        nc.vector.memset(vst[:], 0.0)
        nc.gpsimd.memset(vst[:, :, :, 0, DH : DH + 1], 1.0)
        nc.gpsimd.memset(vst[:, :, :, 1, 0:1], 1.0)

        # PSUM pools: 3x2 banks work ring + 2x1 bank attn accumulators
        work_cm = tc.tile_pool(name="work", bufs=3, space="PSUM")
        work = work_cm.__enter__()
        attnps_cm = tc.tile_pool(name="attnps", bufs=2, space="PSUM")
        attnps = attnps_cm.__enter__()

        def work_tile(name):
            return work.tile([P, 2, 512], F32, tag="work", name=name)

        # ---- K^T projection: kT[pair][2*64 head dims, 1024 keys] ----
        kT_sb = [
            consts.tile([P, NC], F16, tag=f"kT{p}", name=f"kT{p}") for p in range(2)
        ]
        for p in range(2):
            acc = work_tile("kproj_acc")
            for h in range(2):
                for ch in range(DCC):
                    nc.tensor.matmul(
                        acc[:, h, :],
                        wk_sb[:, ch, p * P : (p + 1) * P],
                        ctxT_sb[:, ch, h * 512 : (h + 1) * 512],
                        start=(ch == 0),
                        stop=(ch == DCC - 1),
                    )
            nc.vector.tensor_copy(kT_sb[p][:], acc.rearrange("p a b -> p (a b)"))

        # ---- Q^T projection per 1024-q block (emitted 1 block ahead) ----
        qT_sb = {}

        def emit_qproj(qb):
            xt = xstream.tile([P, DXC, 1024], F16, tag="xt", name="xt")
            nc.sync.dma_start(
                xt[:],
                xT.rearrange("(c p) q -> p c q", p=P)[
                    :, :, qb * 1024 : (qb + 1) * 1024
                ],
            )
            for p in range(2):
                acc = work_tile("qproj_acc")
                for h in range(2):
                    for ch in range(DXC):
                        nc.tensor.matmul(
                            acc[:, h, :],
                            wq_sb[:, ch, p * P : (p + 1) * P],
                            xt[:, ch, h * 512 : (h + 1) * 512],
                            start=(ch == 0),
                            stop=(ch == DXC - 1),
                        )
                qt_t = qtpool.tile([P, 1024], F16, tag="qT", name=f"qT{p}_{qb}")
                qT_sb[(p, qb)] = qt_t
                nc.vector.tensor_copy(qt_t[:], acc.rearrange("p a b -> p (a b)"))

        emit_qproj(0)

        # ---- V projection: keys on partitions, scattered into vst ----
        for kc in range(KC):
            acc = work_tile("vproj_acc")
            av = acc[:, 0, 0:COLS].rearrange("p (a b c) -> p a b c", a=2, b=2)
            for ch in range(DCC):
                nc.tensor.matmul(
                    acc[:, 0, 0:COLS],
                    ctxT_sb[:, ch, kc * P : (kc + 1) * P],
                    wv_sb[:, ch, :],
                    start=(ch == 0),
                    stop=(ch == DCC - 1),
                )
            nc.vector.tensor_copy(vst[:, kc, :, 0, 0:DH], av[:, :, 0, :])
            nc.vector.tensor_copy(vst[:, kc, :, 1, DH:P], av[:, :, 1, :])

        emit_qproj(1)

        ctx_pool_cm.__exit__(None, None, None)

        # ---- attention + output projection, per 512-query tile ----
        attnT_all = {}

        def outproj(qt):
            for sub in range(4):
                o = work_tile("oproj_acc")[:, 0, :]
                for p in range(2):
                    nc.tensor.matmul(
                        o[:],
                        attnT_all[(p, qt)][:, sub * P : (sub + 1) * P],
                        wo_sb[:, p, :],
                        start=(p == 0),
                        stop=(p == 1),
                    )
                ostage = ost.tile([P, DOUT], F32, tag="ostage", name="ostage_t")
                nc.vector.tensor_copy(ostage[:], o[:])
                row = qt * 512 + sub * P
                nc.gpsimd.dma_start(out[row : row + P, :], ostage[:])

        def attn_pair(qt, p):
            qb, qh = qt // 2, qt % 2
            qmv = qT_sb[(p, qb)]
            accA = attnps.tile([P, 512], F32, tag="attnacc", name="accA")
            accB = attnps.tile([P, 512], F32, tag="attnacc", name="accB")
            ex_t = {}

            def emit_sc(kc):
                sc = work_tile("scores_ps")
                for j in range(2):
                    nc.tensor.matmul(
                        sc[:, j, :],
                        kT_sb[p][j * DH : (j + 1) * DH, kc * P : (kc + 1) * P],
                        qmv[j * DH : (j + 1) * DH, qh * 512 : (qh + 1) * 512],
                        start=True,
                        stop=True,
                    )
                ex = etile.tile([P, 2, 512], F16, tag="exp", name="exp_sb")
                nc.scalar.activation(ex[:], sc[:], EXP, scale=SCALE)
                ex_t[kc] = ex

            def emit_av(kc):
                for j in range(2):
                    nc.tensor.matmul(
                        accA[:] if j == 0 else accB[:],
                        vst[:, kc, p, j, :],
                        ex_t[kc][:, j, :],
                        start=(kc == 0),
                        stop=(kc == KC - 1),
                    )

            emit_sc(0)
            emit_sc(1)
            for kc in range(KC):
                if kc + 2 < KC:
                    emit_sc(kc + 2)
                emit_av(kc)

            # ---- DMA-free softmax normalize ----
            # broadcast the RAW denominators across partitions via K=1
            # matmuls, then one 128-lane reciprocal_approx_fast (a [1,512]
            # reciprocal would run on a single DVE lane: ~3.3us)
            d16 = norm.tile([P, 512], F16, tag="den16", name="den16_t")
            nc.vector.tensor_copy(d16[0:1, :], accB[0:1, :])
            nc.vector.tensor_copy(d16[DH : DH + 1, :], accA[DH : DH + 1, :])
            rb = work_tile("rbcast")[:, 0, :]
            nc.tensor.matmul(
                rb[0:DH, :],
                sel[DH : DH + 1, 0:DH],
                d16[DH : DH + 1, :],
                start=True,
                stop=True,
            )
            nc.tensor.matmul(
                rb[DH:P, :],
                sel[0:1, 0:DH],
                d16[0:1, :],
                start=True,
                stop=True,
            )
            rbden = norm.tile([P, 512], F32, tag="rbden", name="rbden_t")
            nc.vector.tensor_copy(rbden[:], rb[:])
            rc = norm.tile([P, 512], F32, tag="rc", name="rc_t")
            nc.vector.reciprocal_approx_fast(rc[:], rbden[:])
            at_t = atpool.tile([P, 512], F16, tag="attnT", name=f"attnT{p}_{qt}")
            attnT_all[(p, qt)] = at_t
            nc.vector.tensor_mul(at_t[0:DH, :], accA[0:DH, :], rc[0:DH, :])
            nc.vector.tensor_mul(at_t[DH:P, :], accB[DH:P, :], rc[DH:P, :])

        for qt in range(NQT):
            attn_pair(qt, 0)
            attn_pair(qt, 1)
            if qt % 2 == 1 and qt // 2 + 2 < NQB:
                emit_qproj(qt // 2 + 2)
            if qt >= 1:
                outproj(qt - 1)
        outproj(NQT - 1)
        attnps_cm.__exit__(None, None, None)
        work_cm.__exit__(None, None, None)


def _build():
    nc = bacc.Bacc(
        "TRN2", target_bir_lowering=False, debug=False, enable_asserts=False
    )
    xT = nc.dram_tensor("xT", [DX, NQ], F16, kind="ExternalInput").ap()
    ctxT = nc.dram_tensor("ctxT", [DC, NC], F16, kind="ExternalInput").ap()
    wq = nc.dram_tensor("wq", [DX, COLS], F16, kind="ExternalInput").ap()
    wk = nc.dram_tensor("wk", [DC, COLS], F16, kind="ExternalInput").ap()
    wv = nc.dram_tensor("wv", [DC, COLS], F16, kind="ExternalInput").ap()
    wo = nc.dram_tensor("wo", [COLS, DOUT], F16, kind="ExternalInput").ap()
    out = nc.dram_tensor("out", [NQ, DOUT], F32, kind="ExternalOutput").ap()
    with tile.TileContext(nc) as tc:
        _emit(tc, nc, xT, ctxT, wq, wk, wv, wo, out)
    nc.compile()
    return nc


_NC = None


def _get_nc():
    global _NC
    if _NC is None:
        _NC = _build()
    return _NC


def _in_maps(x, context, Wq, Wk, Wv, Wo):
    maps = []
    for c in range(8):
        b, g = c // 2, c % 2
        cs = slice(g * COLS, (g + 1) * COLS)
        maps.append(
            {
                "xT": np.ascontiguousarray(x[b].T.astype(F16_NP)),
                "ctxT": np.ascontiguousarray(context[b].T.astype(F16_NP)),
                "wq": np.ascontiguousarray(Wq[:, cs].astype(F16_NP)),
                "wk": np.ascontiguousarray(Wk[:, cs].astype(F16_NP)),
                "wv": np.ascontiguousarray(Wv[:, cs].astype(F16_NP)),
                "wo": np.ascontiguousarray(Wo[cs, :].astype(F16_NP)),
            }
        )
    return maps


def _execute(in_maps, **kw):
    return bass_utils.run_bass_kernel_spmd(
        _get_nc(), in_maps, core_ids=list(range(8)), **kw
    )


def kernel(x, context, Wq, Wk, Wv, Wo, bo):
    x = np.asarray(x, np.float32)
    context = np.asarray(context, np.float32)
    Wq = np.asarray(Wq, np.float32)
    Wk = np.asarray(Wk, np.float32)
    Wv = np.asarray(Wv, np.float32)
    Wo = np.asarray(Wo, np.float32)
    bo = np.asarray(bo, np.float32)
    res = _execute(_in_maps(x, context, Wq, Wk, Wv, Wo))
    parts = [r["out"] for r in res.results]
    out = np.empty((B, NQ, DOUT), np.float32)
    for b in range(B):
        out[b] = parts[2 * b] + parts[2 * b + 1] + bo[None, :]
    return out
